# revision 1
# baseline (speedup 1.0000x reference)
"""Trainium2 Bass kernel for an RNN-T joint network.

Computation (per batch element b):
    enc_p  = enc_out @ W_enc + b_enc                      # (T, J)
    pred_p = pred_out @ W_pred + b_pred                   # (U, J)
    joint  = tanh(enc_p[:, None, :] + pred_p[None, :, :]) # (T, U, J)
    logits = joint @ W_joint + b_joint                    # (T, U, V)

Distribution: data-parallel over the batch dim B=8, one batch element per
NeuronCore.  Inside one core everything is kept J-major (J on the SBUF
partition dim) so that:
  * the broadcast add + tanh fuses into ONE ScalarE activation per
    (j-tile, u): tanh(enc_pT[j, :] + bias) with bias = pred_pT[j, u]
    (a per-partition scalar, which the ACT instruction supports natively),
  * the vocab matmul contraction (over J) has J on the partition dim as
    the tensor engine requires.
Stage-2 (the 17-GFLOP vocab projection) runs in bf16 (fp32 accumulate in
PSUM); stage-1 runs in fp32.  b_joint is added by VectorE while draining
PSUM -> SBUF; output rows are stored u-major so each DMA writes 4KB
contiguous DRAM chunks.
"""

from contextlib import ExitStack

import numpy as np

import concourse.bacc as bacc
import concourse.mybir as mybir
import concourse.tile as tile
import concourse.bass_utils as bass_utils
from concourse.masks import make_identity

N_CORES = 8
T, U, J, V = 256, 64, 512, 1024
DE, DP = 512, 640
KJ = J // 128    # j-tiles
KE = DE // 128   # enc contraction tiles
KP = DP // 128   # pred contraction tiles
HT = T // 128    # t-halves
NV = V // 512    # vocab halves (one PSUM bank each)
F32 = mybir.dt.float32
BF16 = mybir.dt.bfloat16

_CACHE: dict = {}


def _emit(tc, nc, d):
    act = mybir.ActivationFunctionType
    with ExitStack() as ctx:
        const = ctx.enter_context(tc.tile_pool(name="const", bufs=1))
        ps = ctx.enter_context(tc.tile_pool(name="ps", bufs=8, space="PSUM"))
        joint_pool = ctx.enter_context(tc.tile_pool(name="jp", bufs=12))
        out_pool = ctx.enter_context(tc.tile_pool(name="op", bufs=3))

        # ---------------- loads ----------------
        enc_sb = const.tile([128, HT, DE], F32, tag="enc_sb")
        nc.gpsimd.dma_start(enc_sb[:], d["enc_out"].ap().rearrange("(h p) d -> p h d", p=128))
        pred_sb = const.tile([U, DP], F32, tag="pred_sb")
        nc.gpsimd.dma_start(pred_sb[:], d["pred_out"].ap())
        wenc_sb = const.tile([128, KE, J], F32, tag="wenc_sb")
        nc.gpsimd.dma_start(wenc_sb[:], d["W_enc"].ap().rearrange("(k p) j -> p k j", p=128))
        wpred_sb = const.tile([128, KP, J], F32, tag="wpred_sb")
        nc.gpsimd.dma_start(wpred_sb[:], d["W_pred"].ap().rearrange("(k p) j -> p k j", p=128))
        wj_f32 = const.tile([128, KJ, V], F32, tag="wj_f32")
        nc.gpsimd.dma_start(wj_f32[:], d["W_joint"].ap().rearrange("(k p) v -> p k v", p=128))
        wj_sb = const.tile([128, KJ, V], BF16, tag="wj_sb")
        nc.vector.tensor_copy(wj_sb[:], wj_f32[:])

        benc_sb = const.tile([128, KJ], F32, tag="benc_sb")
        nc.gpsimd.dma_start(benc_sb[:], d["b_enc"].ap().rearrange("(k p) -> p k", p=128))
        bpred_sb = const.tile([128, KJ], F32, tag="bpred_sb")
        nc.gpsimd.dma_start(bpred_sb[:], d["b_pred"].ap().rearrange("(k p) -> p k", p=128))
        bsum = const.tile([128, KJ], F32, tag="bsum")
        nc.vector.tensor_add(bsum[:], benc_sb[:], bpred_sb[:])
        # b_joint replicated into every partition (DMA reads it 128x).
        bj_sb = const.tile([128, V], F32, tag="bj_sb")
        nc.sync.dma_start(bj_sb[:], d["b_joint"].ap().unsqueeze(0).to_broadcast((128, V)))

        ident = const.tile([128, 128], F32, tag="ident")
        make_identity(nc, ident[:])

        # ---------------- transposes (PE) ----------------
        # enc_t[dp, dk, t] = enc_out[t, dk*128+dp]
        enc_t = const.tile([128, KE, T], F32, tag="enc_t")
        for k in range(KE):
            pt = ps.tile([128, T], F32, tag="ps", name=f"tr_enc_{k}")
            for h in range(HT):
                nc.tensor.transpose(
                    pt[:, h * 128:(h + 1) * 128],
                    enc_sb[:, h, k * 128:(k + 1) * 128],
                    ident[:],
                )
            nc.scalar.copy(enc_t[:, k, :], pt[:])
        # pred_t[dp, dk, u] = pred_out[u, dk*128+dp]
        pred_t = const.tile([128, KP, U], F32, tag="pred_t")
        for k in range(KP):
            pt = ps.tile([128, U], F32, tag="ps", name=f"tr_pred_{k}")
            nc.tensor.transpose(pt[:], pred_sb[:, k * 128:(k + 1) * 128], ident[:U, :U])
            nc.scalar.copy(pred_t[:, k, :], pt[:])

        # ---------------- stage-1 projections (fp32) ----------------
        # enc_p[jp, jt, t] = (enc_out @ W_enc)[t, jt*128+jp]
        enc_p = const.tile([128, KJ, T], F32, tag="enc_p")
        for j in range(KJ):
            pt = ps.tile([128, T], F32, tag="ps", name=f"mm_enc_{j}")
            for k in range(KE):
                nc.tensor.matmul(
                    pt[:],
                    wenc_sb[:, k, j * 128:(j + 1) * 128],
                    enc_t[:, k, :],
                    start=(k == 0),
                    stop=(k == KE - 1),
                )
            nc.scalar.copy(enc_p[:, j, :], pt[:])
        # pred_p additionally carries b_enc + b_pred (per-partition scalar).
        pred_p = const.tile([128, KJ, U], F32, tag="pred_p")
        for j in range(KJ):
            pt = ps.tile([128, U], F32, tag="ps", name=f"mm_pred_{j}")
            for k in range(KP):
                nc.tensor.matmul(
                    pt[:],
                    wpred_sb[:, k, j * 128:(j + 1) * 128],
                    pred_t[:, k, :],
                    start=(k == 0),
                    stop=(k == KP - 1),
                )
            nc.vector.tensor_scalar_add(pred_p[:, j, :], pt[:], bsum[:, j:j + 1])

        # ---------------- main loop over u ----------------
        out_ap = d["logits"].ap()
        for u in range(U):
            jt = []
            for j in range(KJ):
                jtile = joint_pool.tile([128, T], BF16, tag="joint", name=f"joint_{u}_{j}")
                nc.scalar.activation(
                    jtile[:], enc_p[:, j, :], act.Tanh,
                    bias=pred_p[:, j, u:u + 1], scale=1.0,
                )
                jt.append(jtile)
            ot = out_pool.tile([128, HT, V], F32, tag="out", name=f"out_{u}")
            for h in range(HT):
                for vh in range(NV):
                    pt = ps.tile([128, 512], F32, tag="ps", name=f"mm_{u}_{h}_{vh}")
                    for j in range(KJ):
                        nc.tensor.matmul(
                            pt[:],
                            jt[j][:, h * 128:(h + 1) * 128],
                            wj_sb[:, j, vh * 512:(vh + 1) * 512],
                            start=(j == 0),
                            stop=(j == KJ - 1),
                        )
                    nc.vector.tensor_tensor(
                        ot[:, h, vh * 512:(vh + 1) * 512],
                        pt[:],
                        bj_sb[:, vh * 512:(vh + 1) * 512],
                        mybir.AluOpType.add,
                    )
            nc.sync.dma_start(
                out_ap[:, u, :].rearrange("(h p) v -> p h v", p=128),
                ot[:],
            )


def _build_program():
    nc = bacc.Bacc("TRN2", target_bir_lowering=False, debug=False, num_devices=N_CORES)
    d = {
        "enc_out": nc.dram_tensor("enc_out", (T, DE), F32, kind="ExternalInput"),
        "pred_out": nc.dram_tensor("pred_out", (U, DP), F32, kind="ExternalInput"),
        "W_enc": nc.dram_tensor("W_enc", (DE, J), F32, kind="ExternalInput"),
        "b_enc": nc.dram_tensor("b_enc", (J,), F32, kind="ExternalInput"),
        "W_pred": nc.dram_tensor("W_pred", (DP, J), F32, kind="ExternalInput"),
        "b_pred": nc.dram_tensor("b_pred", (J,), F32, kind="ExternalInput"),
        "W_joint": nc.dram_tensor("W_joint", (J, V), F32, kind="ExternalInput"),
        "b_joint": nc.dram_tensor("b_joint", (V,), F32, kind="ExternalInput"),
        "logits": nc.dram_tensor("logits", (T, U, V), F32, kind="ExternalOutput"),
    }
    with tile.TileContext(nc) as tc:
        _emit(tc, nc, d)
    nc.compile()
    return nc


def kernel(enc_out, pred_out, W_enc, b_enc, W_pred, b_pred, W_joint, b_joint):
    nc = _CACHE.get("nc")
    if nc is None:
        nc = _CACHE["nc"] = _build_program()

    shared = {
        "W_enc": np.ascontiguousarray(W_enc, dtype=np.float32),
        "b_enc": np.ascontiguousarray(b_enc, dtype=np.float32),
        "W_pred": np.ascontiguousarray(W_pred, dtype=np.float32),
        "b_pred": np.ascontiguousarray(b_pred, dtype=np.float32),
        "W_joint": np.ascontiguousarray(W_joint, dtype=np.float32),
        "b_joint": np.ascontiguousarray(b_joint, dtype=np.float32),
    }
    in_maps = [
        {
            "enc_out": np.ascontiguousarray(enc_out[c], dtype=np.float32),
            "pred_out": np.ascontiguousarray(pred_out[c], dtype=np.float32),
            **shared,
        }
        for c in range(N_CORES)
    ]
    res = bass_utils.run_bass_kernel_spmd(nc, in_maps, core_ids=list(range(N_CORES)))
    _CACHE["last_results"] = res
    return np.stack([res.results[c]["logits"] for c in range(N_CORES)])


# revision 5
# speedup vs baseline: 106349.0450x; 106349.0450x over previous
"""Trainium2 Bass kernel for an RNN-T joint network.

Computation (per batch element b):
    enc_p  = enc_out @ W_enc + b_enc                      # (T, J)
    pred_p = pred_out @ W_pred + b_pred                   # (U, J)
    joint  = tanh(enc_p[:, None, :] + pred_p[None, :, :]) # (T, U, J)
    logits = joint @ W_joint + b_joint                    # (T, U, V)

Distribution: data-parallel over the batch dim B=8, one batch element per
NeuronCore.  Inside one core everything is kept J-major (J on the SBUF
partition dim) so that:
  * the broadcast add + tanh fuses into ONE ScalarE activation per
    (j-tile, u): tanh(enc_pT[j, :] + bias) with bias = pred_pT[j, u]
    (a per-partition scalar, which the ACT instruction supports natively),
  * the vocab matmul contraction (over J) has J on the partition dim as
    the tensor engine requires.
Stage-2 (the 17-GFLOP vocab projection) runs in bf16 (fp32 accumulate in
PSUM); stage-1 runs in fp32.  b_joint is added by VectorE while draining
PSUM -> SBUF; output rows are stored u-major so each DMA writes 4KB
contiguous DRAM chunks.
"""

from contextlib import ExitStack

import numpy as np

import concourse.bacc as bacc
import concourse.mybir as mybir
import concourse.tile as tile
import concourse.bass_utils as bass_utils
from concourse.masks import make_identity

N_CORES = 8
T, U, J, V = 256, 64, 512, 1024
DE, DP = 512, 640
KJ = J // 128    # j-tiles
KE = DE // 128   # enc contraction tiles
KP = DP // 128   # pred contraction tiles
HT = T // 128    # t-halves
NV = V // 512    # vocab halves (one PSUM bank each)
F32 = mybir.dt.float32
BF16 = mybir.dt.bfloat16

_CACHE: dict = {}


def _emit(tc, nc, d, repeats=1):
    act = mybir.ActivationFunctionType
    with ExitStack() as ctx:
        const = ctx.enter_context(tc.tile_pool(name="const", bufs=1))
        stg = ctx.enter_context(tc.tile_pool(name="stg", bufs=1 if repeats == 1 else 2))
        ps = ctx.enter_context(tc.tile_pool(name="ps", bufs=8, space="PSUM"))
        joint_pool = ctx.enter_context(tc.tile_pool(name="jp", bufs=12))
        out_pool = ctx.enter_context(tc.tile_pool(name="op", bufs=3))

        # ---------------- loads ----------------
        enc_sb = const.tile([128, HT, DE], F32, tag="enc_sb")
        nc.gpsimd.dma_start(enc_sb[:], d["enc_out"].ap().rearrange("(h p) d -> p h d", p=128))
        pred_sb = const.tile([U, DP], F32, tag="pred_sb")
        nc.gpsimd.dma_start(pred_sb[:], d["pred_out"].ap())
        wenc_sb = const.tile([128, KE, J], F32, tag="wenc_sb")
        nc.gpsimd.dma_start(wenc_sb[:], d["W_enc"].ap().rearrange("(k p) j -> p k j", p=128))
        wpred_sb = const.tile([128, KP, J], F32, tag="wpred_sb")
        nc.gpsimd.dma_start(wpred_sb[:], d["W_pred"].ap().rearrange("(k p) j -> p k j", p=128))
        wj_f32 = const.tile([128, KJ, V], F32, tag="wj_f32")
        nc.gpsimd.dma_start(wj_f32[:], d["W_joint"].ap().rearrange("(k p) v -> p k v", p=128))
        wj_sb = const.tile([128, KJ, V], BF16, tag="wj_sb")
        nc.vector.tensor_copy(wj_sb[:], wj_f32[:])

        benc_sb = const.tile([128, KJ], F32, tag="benc_sb")
        nc.gpsimd.dma_start(benc_sb[:], d["b_enc"].ap().rearrange("(k p) -> p k", p=128))
        bpred_sb = const.tile([128, KJ], F32, tag="bpred_sb")
        nc.gpsimd.dma_start(bpred_sb[:], d["b_pred"].ap().rearrange("(k p) -> p k", p=128))
        bsum = const.tile([128, KJ], F32, tag="bsum")
        nc.vector.tensor_add(bsum[:], benc_sb[:], bpred_sb[:])
        # b_joint replicated into every partition (DMA reads it 128x).
        bj_sb = const.tile([128, V], F32, tag="bj_sb")
        nc.sync.dma_start(bj_sb[:], d["b_joint"].ap().unsqueeze(0).to_broadcast((128, V)))

        ident = const.tile([128, 128], F32, tag="ident")
        make_identity(nc, ident[:])

        out_ap = d["logits"].ap()
        for rep in range(repeats):
            # ---------------- transposes (PE) ----------------
            # enc_t[dp, dk, t] = enc_out[t, dk*128+dp]
            enc_t = stg.tile([128, KE, T], F32, tag="enc_t", name=f"enc_t_{rep}")
            for k in range(KE):
                pt = ps.tile([128, T], F32, tag="ps", name=f"tr_enc_{rep}_{k}")
                for h in range(HT):
                    nc.tensor.transpose(
                        pt[:, h * 128:(h + 1) * 128],
                        enc_sb[:, h, k * 128:(k + 1) * 128],
                        ident[:],
                    )
                nc.scalar.copy(enc_t[:, k, :], pt[:])
            # pred_t[dp, dk, u] = pred_out[u, dk*128+dp]
            pred_t = stg.tile([128, KP, U], F32, tag="pred_t", name=f"pred_t_{rep}")
            for k in range(KP):
                pt = ps.tile([128, U], F32, tag="ps", name=f"tr_pred_{rep}_{k}")
                nc.tensor.transpose(pt[:], pred_sb[:, k * 128:(k + 1) * 128], ident[:U, :U])
                nc.scalar.copy(pred_t[:, k, :], pt[:])

            # ---------------- stage-1 projections (fp32) ----------------
            # enc_p[jp, jt, t] = (enc_out @ W_enc)[t, jt*128+jp]
            enc_p = stg.tile([128, KJ, T], F32, tag="enc_p", name=f"enc_p_{rep}")
            for j in range(KJ):
                pt = ps.tile([128, T], F32, tag="ps", name=f"mm_enc_{rep}_{j}")
                for k in range(KE):
                    nc.tensor.matmul(
                        pt[:],
                        wenc_sb[:, k, j * 128:(j + 1) * 128],
                        enc_t[:, k, :],
                        start=(k == 0),
                        stop=(k == KE - 1),
                    )
                nc.scalar.copy(enc_p[:, j, :], pt[:])
            # pred_p additionally carries b_enc + b_pred (per-partition scalar).
            pred_p = stg.tile([128, KJ, U], F32, tag="pred_p", name=f"pred_p_{rep}")
            for j in range(KJ):
                pt = ps.tile([128, U], F32, tag="ps", name=f"mm_pred_{rep}_{j}")
                for k in range(KP):
                    nc.tensor.matmul(
                        pt[:],
                        wpred_sb[:, k, j * 128:(j + 1) * 128],
                        pred_t[:, k, :],
                        start=(k == 0),
                        stop=(k == KP - 1),
                    )
                nc.vector.tensor_scalar_add(pred_p[:, j, :], pt[:], bsum[:, j:j + 1])

            # ---------------- main loop over u ----------------
            for u in range(U):
                jt = []
                for j in range(KJ):
                    jtile = joint_pool.tile([128, T], BF16, tag="joint", name=f"joint_{rep}_{u}_{j}")
                    nc.scalar.activation(
                        jtile[:], enc_p[:, j, :], act.Tanh,
                        bias=pred_p[:, j, u:u + 1], scale=1.0,
                    )
                    jt.append(jtile)
                ot = out_pool.tile([128, HT, V], F32, tag="out", name=f"out_{rep}_{u}")
                for h in range(HT):
                    for vh in range(NV):
                        pt = ps.tile([128, 512], F32, tag="ps", name=f"mm_{rep}_{u}_{h}_{vh}")
                        for j in range(KJ):
                            nc.tensor.matmul(
                                pt[:],
                                jt[j][:, h * 128:(h + 1) * 128],
                                wj_sb[:, j, vh * 512:(vh + 1) * 512],
                                start=(j == 0),
                                stop=(j == KJ - 1),
                            )
                        nc.vector.tensor_tensor(
                            ot[:, h, vh * 512:(vh + 1) * 512],
                            pt[:],
                            bj_sb[:, vh * 512:(vh + 1) * 512],
                            mybir.AluOpType.add,
                        )
                nc.sync.dma_start(
                    out_ap[:, u, :].rearrange("(h p) v -> p h v", p=128),
                    ot[:],
                )


def _build_program(repeats=1):
    nc = bacc.Bacc("TRN2", target_bir_lowering=False, debug=False, num_devices=N_CORES)
    d = {
        "enc_out": nc.dram_tensor("enc_out", (T, DE), F32, kind="ExternalInput"),
        "pred_out": nc.dram_tensor("pred_out", (U, DP), F32, kind="ExternalInput"),
        "W_enc": nc.dram_tensor("W_enc", (DE, J), F32, kind="ExternalInput"),
        "b_enc": nc.dram_tensor("b_enc", (J,), F32, kind="ExternalInput"),
        "W_pred": nc.dram_tensor("W_pred", (DP, J), F32, kind="ExternalInput"),
        "b_pred": nc.dram_tensor("b_pred", (J,), F32, kind="ExternalInput"),
        "W_joint": nc.dram_tensor("W_joint", (J, V), F32, kind="ExternalInput"),
        "b_joint": nc.dram_tensor("b_joint", (V,), F32, kind="ExternalInput"),
        "logits": nc.dram_tensor("logits", (T, U, V), F32, kind="ExternalOutput"),
    }
    with tile.TileContext(nc) as tc:
        _emit(tc, nc, d, repeats=repeats)
    nc.compile()
    return nc


def kernel(enc_out, pred_out, W_enc, b_enc, W_pred, b_pred, W_joint, b_joint):
    nc = _CACHE.get("nc")
    if nc is None:
        nc = _CACHE["nc"] = _build_program()

    shared = {
        "W_enc": np.ascontiguousarray(W_enc, dtype=np.float32),
        "b_enc": np.ascontiguousarray(b_enc, dtype=np.float32),
        "W_pred": np.ascontiguousarray(W_pred, dtype=np.float32),
        "b_pred": np.ascontiguousarray(b_pred, dtype=np.float32),
        "W_joint": np.ascontiguousarray(W_joint, dtype=np.float32),
        "b_joint": np.ascontiguousarray(b_joint, dtype=np.float32),
    }
    in_maps = [
        {
            "enc_out": np.ascontiguousarray(enc_out[c], dtype=np.float32),
            "pred_out": np.ascontiguousarray(pred_out[c], dtype=np.float32),
            **shared,
        }
        for c in range(N_CORES)
    ]
    res = bass_utils.run_bass_kernel_spmd(nc, in_maps, core_ids=list(range(N_CORES)))
    _CACHE["last_results"] = res
    return np.stack([res.results[c]["logits"] for c in range(N_CORES)])
